# revision 6
# baseline (speedup 1.0000x reference)
"""Cross-attention Bass/Tile kernel for Trainium2, data-parallel over batch on 8 cores.

Problem (hardcoded): x_1 [2048,16,100], x_2 [2048,16,100], Wq/Wk/Wv [100,128], fp32.
  Q = x1 @ Wq; K = x2 @ Wk; V = x2 @ Wv  (per batch)
  out = softmax(Q K^T / sqrt(128)) @ V   -> [2048,16,128]

Sharding: batch dim split 8 ways (2 batches per core). Full inputs in, full output out.

Per-core dataflow (2 batches), bf16 matmul core:
  - x tiles loaded with gpsimd cast-DMA (fp32->bf16), DMA-transposed (xbar) -> x^T bf16
  - Q^T,K^T,V^T = W^T @ x^T bf16 matmuls -> bf16 [128,2048] (k/v on partitions)
  - V tiles [t,v] from V^T via DMA-transpose
  - per 512-chunk of s:
      S^T tiles [t=128,s=512] = K^T_tile.T @ Q^T_chunk  (bf16), pairs in [128,1024] psum
      E^T = exp(S^T/sqrt(dk)) on ACT -> bf16
      rowsum: K_PE tiles via PE ones-matmuls -> [1,512] psum; rest via DVE adds
        -> rowacc [128,512]; per 128-block combined into [s,1] via two matmuls
      O^T[v,s] += V_tile.T @ E^T_tile  (bf16, psum-accumulated)
      O^T -> SBUF (ACT), PE-transpose 128-blocks -> O [s,v], normalize fused into
      ACT psum->SBUF copy (scale=1/rowsum per-partition), DMA out
"""

import sys

sys.path.insert(0, "/opt/trn_rl_repo")

import numpy as np

import concourse.bass as bass
import concourse.tile as tile
from concourse import bacc, mybir
from concourse.bass_utils import run_bass_kernel_spmd
from concourse.masks import make_identity

S = 2048          # sequence length (both sides)
B = 16            # total batch
DH = 100          # input feature dim
DK = 128          # head dim
NCORES = 8
BPC = B // NCORES  # batches per core
F32 = mybir.dt.float32
BF16 = mybir.dt.bfloat16
SCALE = 1.0 / float(np.sqrt(np.float32(DK)))

ST = S // 128     # 16 t-tiles of 128
NSC = S // 512    # 4 chunks of 512
K_PE = 4          # t-tiles whose rowsum contribution is computed on PE (rest on DVE)


def _attention_kernel(tc, out, x1, x2, wq, wk, wv):
    nc = tc.nc

    with (
        tc.tile_pool(name="const", bufs=1) as constp,
        tc.tile_pool(name="xn", bufs=6) as xnp,
        tc.tile_pool(name="xT", bufs=4) as xtp,
        tc.tile_pool(name="qk", bufs=4) as qkp,
        tc.tile_pool(name="vt", bufs=2) as vtp,
        tc.tile_pool(name="vp", bufs=2) as vp,
        tc.tile_pool(name="et", bufs=2) as etp,
        tc.tile_pool(name="row", bufs=2) as rowp,
        tc.tile_pool(name="rsb", bufs=2) as rsbp,
        tc.tile_pool(name="rr", bufs=4) as rrp,
        tc.tile_pool(name="osb", bufs=3) as osbp,
        tc.tile_pool(name="ps_big", bufs=2, space="PSUM") as psb,
        tc.tile_pool(name="ps_ot", bufs=2, space="PSUM") as psot,
        tc.tile_pool(name="ps_sm", bufs=2, space="PSUM") as pssm,
    ):
        ident = constp.tile([128, 128], F32)
        make_identity(nc, ident)
        ones_bf = constp.tile([128, 1], BF16)
        nc.vector.memset(ones_bf, 1.0)
        ones_f32 = constp.tile([128, 1], F32)
        nc.vector.memset(ones_f32, 1.0)
        w_sbs = []
        for wap, wname in ((wq, "wq"), (wk, "wk"), (wv, "wv")):
            w_f32 = constp.tile([DH, DK], F32, name=f"{wname}_f32")
            nc.sync.dma_start(w_f32, wap)
            w_sb = constp.tile([DH, DK], BF16, name=f"{wname}_sb")
            nc.vector.tensor_copy(w_sb, w_f32)
            w_sbs.append(w_sb)
        wq_sb, wk_sb, wv_sb = w_sbs

        # ---- x^T [d, S] bf16 per (input, batch): cast-DMA load + DMA-transpose.
        # One [128, 2*DH] load covers both batches (contiguous rows); the
        # transpose view for batch b is cols [b*DH, b*DH+128) (128-wide, valid).
        xTs = {}
        for src_i, src_ap in ((0, x1), (1, x2)):
            for b in range(BPC):
                xTs[(src_i, b)] = xtp.tile(
                    [128, S], BF16, tag="xT", name=f"xT_{src_i}_{b}"
                )
        for src_i, src_ap in ((0, x1), (1, x2)):
            for st in range(ST):
                xn2 = xnp.tile([128, 256], BF16, tag="xn", name=f"xn_{src_i}_{st}")
                nc.gpsimd.dma_start(
                    xn2[:, : BPC * DH],
                    src_ap[st * 128:(st + 1) * 128, :, :],
                )
                nc.vector.memset(xn2[:, BPC * DH:], 0.0)
                for b in range(BPC):
                    nc.sync.dma_start(
                        xTs[(src_i, b)][:, st * 128:(st + 1) * 128],
                        xn2[:, b * DH: b * DH + 128],
                        transpose=True,
                    )

        for b in range(BPC):
            x1T = xTs[(0, b)]
            x2T = xTs[(1, b)]
            # ---- projections (bf16): Q^T, K^T, V^T [DK, S]
            qT = qkp.tile([DK, S], BF16, tag="qk", name=f"qT_{b}")
            kT = qkp.tile([DK, S], BF16, tag="qk", name=f"kT_{b}")
            vTsb = vtp.tile([DK, S], BF16, tag="vt", name=f"vT_{b}")
            for dstT, w_sb, xT in (
                (qT, wq_sb, x1T), (kT, wk_sb, x2T), (vTsb, wv_sb, x2T)
            ):
                for c in range(NSC):
                    csl = slice(c * 512, (c + 1) * 512)
                    ps = psot.tile([DK, 512], F32, tag="ot", name=f"pj_{b}_{c}")
                    nc.tensor.matmul(ps, w_sb, xT[:DH, csl], start=True, stop=True)
                    nc.scalar.copy(dstT[:, csl], ps)
            # V tiles [t, v] bf16 via DMA-transpose of V^T
            vall = vp.tile([128, S], BF16, tag="v", name=f"vall_{b}")
            for tt in range(ST):
                tsl = slice(tt * 128, (tt + 1) * 128)
                nc.sync.dma_start(vall[:, tsl], vTsb[:, tsl], transpose=True)

            # ---- attention, per 512-wide chunk of s
            for sc in range(NSC):
                ssl = slice(sc * 512, (sc + 1) * 512)
                # S^T tile pairs -> one [128,1024] psum tile -> exp -> E^T bf16
                et_all = etp.tile([128, ST * 512], BF16, tag="et", name=f"et_{b}_{sc}")
                for tp2 in range(ST // 2):
                    ps = psb.tile([128, 1024], F32, tag="big", name=f"st_{b}_{sc}_{tp2}")
                    for h in range(2):
                        tt = tp2 * 2 + h
                        nc.tensor.matmul(
                            ps[:, h * 512:(h + 1) * 512],
                            kT[:, tt * 128:(tt + 1) * 128],
                            qT[:, ssl],
                            start=True, stop=True,
                        )
                    nc.scalar.activation(
                        et_all[:, tp2 * 1024:(tp2 + 1) * 1024], ps,
                        mybir.ActivationFunctionType.Exp, scale=SCALE,
                    )
                # rowsum part 1: first K_PE tiles via PE ones-matmuls -> rsps [1,512]
                rsps = pssm.tile([1, 512], F32, tag="sm", name=f"rsps_{b}_{sc}")
                for tt in range(K_PE):
                    nc.tensor.matmul(
                        rsps, ones_bf, et_all[:, tt * 512:(tt + 1) * 512],
                        start=(tt == 0), stop=(tt == K_PE - 1),
                    )
                rsps_sb = rsbp.tile([1, 512], F32, tag="rsb", name=f"rsb_{b}_{sc}")
                nc.vector.tensor_copy(rsps_sb, rsps)
                # rowsum part 2: remaining tiles via DVE adds -> rowacc [128,512]
                rowacc = rowp.tile([128, 512], F32, tag="row", name=f"row_{b}_{sc}")
                nc.vector.tensor_add(
                    rowacc,
                    et_all[:, K_PE * 512:(K_PE + 1) * 512],
                    et_all[:, (K_PE + 1) * 512:(K_PE + 2) * 512],
                )
                for tt in range(K_PE + 2, ST):
                    nc.vector.tensor_add(
                        rowacc, rowacc, et_all[:, tt * 512:(tt + 1) * 512]
                    )
                # O^T [v, s] accumulation over t tiles (bf16)
                otp = psot.tile([128, 512], F32, tag="ot", name=f"ot_{b}_{sc}")
                for tt in range(ST):
                    nc.tensor.matmul(
                        otp,
                        vall[:, tt * 128:(tt + 1) * 128],
                        et_all[:, tt * 512:(tt + 1) * 512],
                        start=(tt == 0),
                        stop=(tt == ST - 1),
                    )
                ot_sb = osbp.tile([128, 512], F32, tag="osb", name=f"otsb_{b}_{sc}")
                nc.scalar.copy(ot_sb, otp)
                # per 128-block: rowsum [s,1] = rowacc.T@ones + rsps slice (2 matmuls),
                # recip, transpose O^T -> O, normalize fused into ACT copy, DMA out
                for si in range(4):
                    scol = slice(si * 128, (si + 1) * 128)
                    rs = pssm.tile([128, 1], F32, tag="sm", name=f"rs_{b}_{sc}_{si}")
                    nc.tensor.matmul(rs, rowacc[:, scol], ones_f32,
                                     start=True, stop=False)
                    nc.tensor.matmul(rs, rsps_sb[:, scol], ones_f32[:1, :],
                                     start=False, stop=True)
                    rr = rrp.tile([128, 1], F32, tag="rr", name=f"rr_{b}_{sc}_{si}")
                    nc.vector.reciprocal(rr, rs)
                    otr = pssm.tile([128, 128], F32, tag="sm", name=f"otr_{b}_{sc}_{si}")
                    nc.tensor.transpose(otr, ot_sb[:, scol], ident)
                    osc = osbp.tile([128, DK], F32, tag="osc", name=f"osc_{b}_{sc}_{si}")
                    nc.scalar.mul(osc, otr, rr)
                    s0 = sc * 512 + si * 128
                    nc.sync.dma_start(out[s0:s0 + 128, b, :], osc)


_NC_CACHE = None


def _build():
    global _NC_CACHE
    if _NC_CACHE is not None:
        return _NC_CACHE
    nc = bacc.Bacc("TRN2", target_bir_lowering=False, debug=False, num_devices=NCORES)
    x1 = nc.dram_tensor("x_1", (S, BPC, DH), F32, kind="ExternalInput").ap()
    x2 = nc.dram_tensor("x_2", (S, BPC, DH), F32, kind="ExternalInput").ap()
    wq = nc.dram_tensor("Wq", (DH, DK), F32, kind="ExternalInput").ap()
    wk = nc.dram_tensor("Wk", (DH, DK), F32, kind="ExternalInput").ap()
    wv = nc.dram_tensor("Wv", (DH, DK), F32, kind="ExternalInput").ap()
    out = nc.dram_tensor("out", (S, BPC, DK), F32, kind="ExternalOutput").ap()
    with tile.TileContext(nc) as tc:
        _attention_kernel(tc, out, x1, x2, wq, wk, wv)
    nc.compile()
    _NC_CACHE = nc
    return nc


def _in_maps(x_1, x_2, Wq, Wk, Wv):
    maps = []
    for c in range(NCORES):
        bsl = slice(c * BPC, (c + 1) * BPC)
        maps.append({
            "x_1": np.ascontiguousarray(x_1[:, bsl, :], dtype=np.float32),
            "x_2": np.ascontiguousarray(x_2[:, bsl, :], dtype=np.float32),
            "Wq": np.asarray(Wq, dtype=np.float32),
            "Wk": np.asarray(Wk, dtype=np.float32),
            "Wv": np.asarray(Wv, dtype=np.float32),
        })
    return maps


def run(x_1, x_2, Wq, Wk, Wv, **spmd_kwargs):
    nc = _build()
    res = run_bass_kernel_spmd(
        nc, _in_maps(x_1, x_2, Wq, Wk, Wv), core_ids=list(range(NCORES)), **spmd_kwargs
    )
    out = np.concatenate([res.results[c]["out"] for c in range(NCORES)], axis=1)
    return out, res


def kernel(x_1, x_2, Wq, Wk, Wv):
    out, _ = run(x_1, x_2, Wq, Wk, Wv)
    return out.astype(np.float32)


# revision 7
# speedup vs baseline: 2.2135x; 2.2135x over previous
"""Cross-attention Bass/Tile kernel for Trainium2, data-parallel over batch on 8 cores.

Problem (hardcoded): x_1 [2048,16,100], x_2 [2048,16,100], Wq/Wk/Wv [100,128], fp32.
  Q = x1 @ Wq; K = x2 @ Wk; V = x2 @ Wv  (per batch)
  out = softmax(Q K^T / sqrt(128)) @ V   -> [2048,16,128]

Sharding: batch dim split 8 ways (2 batches per core). Full inputs in, full output out.

Per-core dataflow (2 batches), bf16 matmul core:
  - x tiles loaded with gpsimd cast-DMA (fp32->bf16), DMA-transposed (xbar) -> x^T bf16
  - Q^T,K^T,V^T = W^T @ x^T bf16 matmuls -> bf16 [128,2048] (k/v on partitions)
  - V tiles [t,v] from V^T via DMA-transpose
  - per 512-chunk of s:
      S^T tiles [t=128,s=512] = K^T_tile.T @ Q^T_chunk  (bf16), pairs in [128,1024] psum
      E^T = exp(S^T/sqrt(dk)) on ACT -> bf16
      rowsum: K_PE tiles via PE ones-matmuls -> [1,512] psum; rest via DVE adds
        -> rowacc [128,512]; per 128-block combined into [s,1] via two matmuls
      O^T[v,s] += V_tile.T @ E^T_tile  (bf16, psum-accumulated)
      O^T -> SBUF (ACT), PE-transpose 128-blocks -> O [s,v], normalize fused into
      ACT psum->SBUF copy (scale=1/rowsum per-partition), DMA out
"""

import sys

sys.path.insert(0, "/opt/trn_rl_repo")

import numpy as np

import concourse.bass as bass
import concourse.tile as tile
from concourse import bacc, mybir
from concourse.bass_utils import run_bass_kernel_spmd
from concourse.masks import make_identity

S = 2048          # sequence length (both sides)
B = 16            # total batch
DH = 100          # input feature dim
DK = 128          # head dim
NCORES = 8
BPC = B // NCORES  # batches per core
F32 = mybir.dt.float32
BF16 = mybir.dt.bfloat16
SCALE = 1.0 / float(np.sqrt(np.float32(DK)))

ST = S // 128     # 16 t-tiles of 128
NSC = S // 512    # 4 chunks of 512
K_PE = 4          # t-tiles whose rowsum contribution is computed on PE (rest on DVE)


def _attention_kernel(tc, out, x1, x2, wq, wk, wv):
    nc = tc.nc

    with (
        tc.tile_pool(name="const", bufs=1) as constp,
        tc.tile_pool(name="xn", bufs=6) as xnp,
        tc.tile_pool(name="xT", bufs=4) as xtp,
        tc.tile_pool(name="qk", bufs=4) as qkp,
        tc.tile_pool(name="vt", bufs=2) as vtp,
        tc.tile_pool(name="vp", bufs=2) as vp,
        tc.tile_pool(name="et", bufs=2) as etp,
        tc.tile_pool(name="row", bufs=2) as rowp,
        tc.tile_pool(name="rsb", bufs=2) as rsbp,
        tc.tile_pool(name="rr", bufs=4) as rrp,
        tc.tile_pool(name="osb", bufs=3) as osbp,
        tc.tile_pool(name="ps_big", bufs=2, space="PSUM") as psb,
        tc.tile_pool(name="ps_ot", bufs=2, space="PSUM") as psot,
        tc.tile_pool(name="ps_sm", bufs=2, space="PSUM") as pssm,
    ):
        ident = constp.tile([128, 128], F32)
        make_identity(nc, ident)
        ident_bf = constp.tile([128, 128], BF16)
        nc.vector.tensor_copy(ident_bf, ident)
        ones_bf = constp.tile([128, 1], BF16)
        nc.vector.memset(ones_bf, 1.0)
        ones_f32 = constp.tile([128, 1], F32)
        nc.vector.memset(ones_f32, 1.0)
        w_sbs = []
        for wap, wname in ((wq, "wq"), (wk, "wk"), (wv, "wv")):
            w_f32 = constp.tile([DH, DK], F32, name=f"{wname}_f32")
            nc.sync.dma_start(w_f32, wap)
            w_sb = constp.tile([DH, DK], BF16, name=f"{wname}_sb")
            nc.vector.tensor_copy(w_sb, w_f32)
            w_sbs.append(w_sb)
        wq_sb, wk_sb, wv_sb = w_sbs

        # ---- x^T [d, S] bf16 per (input, batch): cast-DMA load + DMA-transpose.
        # One [128, 2*DH] load covers both batches (contiguous rows); the
        # transpose view for batch b is cols [b*DH, b*DH+128) (128-wide, valid).
        xTs = {}
        for src_i, src_ap in ((0, x1), (1, x2)):
            for b in range(BPC):
                xTs[(src_i, b)] = xtp.tile(
                    [128, S], BF16, tag="xT", name=f"xT_{src_i}_{b}"
                )
        for src_i, src_ap in ((0, x1), (1, x2)):
            for st in range(ST):
                xn2 = xnp.tile([128, 256], BF16, tag="xn", name=f"xn_{src_i}_{st}")
                nc.gpsimd.dma_start(
                    xn2[:, : BPC * DH],
                    src_ap[st * 128:(st + 1) * 128, :, :],
                )
                nc.vector.memset(xn2[:, BPC * DH:], 0.0)
                for b in range(BPC):
                    xtps = pssm.tile([128, 128], BF16, tag="sm",
                                     name=f"xtp_{src_i}_{st}_{b}")
                    nc.tensor.transpose(
                        xtps, xn2[:, b * DH: b * DH + 128], ident_bf
                    )
                    nc.scalar.copy(
                        xTs[(src_i, b)][:, st * 128:(st + 1) * 128], xtps
                    )

        for b in range(BPC):
            x1T = xTs[(0, b)]
            x2T = xTs[(1, b)]
            # ---- projections (bf16): Q^T, K^T, V^T [DK, S]
            qT = qkp.tile([DK, S], BF16, tag="qk", name=f"qT_{b}")
            kT = qkp.tile([DK, S], BF16, tag="qk", name=f"kT_{b}")
            vTsb = vtp.tile([DK, S], BF16, tag="vt", name=f"vT_{b}")
            for dstT, w_sb, xT in (
                (qT, wq_sb, x1T), (kT, wk_sb, x2T), (vTsb, wv_sb, x2T)
            ):
                for c in range(NSC):
                    csl = slice(c * 512, (c + 1) * 512)
                    ps = psot.tile([DK, 512], F32, tag="ot", name=f"pj_{b}_{c}")
                    nc.tensor.matmul(ps, w_sb, xT[:DH, csl], start=True, stop=True)
                    nc.vector.tensor_copy(dstT[:, csl], ps)
            # V tiles [t, v] bf16 via DMA-transpose of V^T
            vall = vp.tile([128, S], BF16, tag="v", name=f"vall_{b}")
            for tt in range(ST):
                tsl = slice(tt * 128, (tt + 1) * 128)
                vtps = pssm.tile([128, 128], BF16, tag="sm", name=f"vtp_{b}_{tt}")
                nc.tensor.transpose(vtps, vTsb[:, tsl], ident_bf)
                nc.scalar.copy(vall[:, tsl], vtps)

            # ---- attention, per 512-wide chunk of s
            for sc in range(NSC):
                ssl = slice(sc * 512, (sc + 1) * 512)
                # S^T tile pairs -> one [128,1024] psum tile -> exp -> E^T bf16
                et_all = etp.tile([128, ST * 512], BF16, tag="et", name=f"et_{b}_{sc}")
                for tp2 in range(ST // 2):
                    ps = psb.tile([128, 1024], F32, tag="big", name=f"st_{b}_{sc}_{tp2}")
                    for h in range(2):
                        tt = tp2 * 2 + h
                        nc.tensor.matmul(
                            ps[:, h * 512:(h + 1) * 512],
                            kT[:, tt * 128:(tt + 1) * 128],
                            qT[:, ssl],
                            start=True, stop=True,
                        )
                    nc.scalar.activation(
                        et_all[:, tp2 * 1024:(tp2 + 1) * 1024], ps,
                        mybir.ActivationFunctionType.Exp, scale=SCALE,
                    )
                # rowsum: tree-structured DVE adds (wide views halve each level)
                acc = rowp.tile([128, 4096], F32, tag="acc", name=f"acc_{b}_{sc}")
                nc.vector.tensor_add(acc, et_all[:, :4096], et_all[:, 4096:])
                nc.vector.tensor_add(acc[:, :2048], acc[:, :2048], acc[:, 2048:])
                nc.vector.tensor_add(acc[:, :1024], acc[:, :1024], acc[:, 1024:2048])
                rowacc = acc[:, :512]
                nc.vector.tensor_add(rowacc, rowacc, acc[:, 512:1024])
                # O^T [v, s] accumulation over t tiles (bf16)
                otp = psot.tile([128, 512], F32, tag="ot", name=f"ot_{b}_{sc}")
                for tt in range(ST):
                    nc.tensor.matmul(
                        otp,
                        vall[:, tt * 128:(tt + 1) * 128],
                        et_all[:, tt * 512:(tt + 1) * 512],
                        start=(tt == 0),
                        stop=(tt == ST - 1),
                    )
                ot_sb = osbp.tile([128, 512], F32, tag="osb", name=f"otsb_{b}_{sc}")
                nc.scalar.copy(ot_sb, otp)
                # per 128-block: rowsum [s,1] = rowacc.T@ones + rsps slice (2 matmuls),
                # recip, transpose O^T -> O, normalize fused into ACT copy, DMA out
                for si in range(4):
                    scol = slice(si * 128, (si + 1) * 128)
                    rs = pssm.tile([128, 1], F32, tag="sm", name=f"rs_{b}_{sc}_{si}")
                    nc.tensor.matmul(rs, rowacc[:, scol], ones_f32,
                                     start=True, stop=True)
                    rr = rrp.tile([128, 1], F32, tag="rr", name=f"rr_{b}_{sc}_{si}")
                    nc.vector.reciprocal(rr, rs)
                    otr = pssm.tile([128, 128], F32, tag="sm", name=f"otr_{b}_{sc}_{si}")
                    nc.tensor.transpose(otr, ot_sb[:, scol], ident)
                    osc = osbp.tile([128, DK], F32, tag="osc", name=f"osc_{b}_{sc}_{si}")
                    nc.scalar.mul(osc, otr, rr)
                    s0 = sc * 512 + si * 128
                    nc.sync.dma_start(out[s0:s0 + 128, b, :], osc)


_NC_CACHE = None


def _build():
    global _NC_CACHE
    if _NC_CACHE is not None:
        return _NC_CACHE
    nc = bacc.Bacc("TRN2", target_bir_lowering=False, debug=False, num_devices=NCORES)
    x1 = nc.dram_tensor("x_1", (S, BPC, DH), F32, kind="ExternalInput").ap()
    x2 = nc.dram_tensor("x_2", (S, BPC, DH), F32, kind="ExternalInput").ap()
    wq = nc.dram_tensor("Wq", (DH, DK), F32, kind="ExternalInput").ap()
    wk = nc.dram_tensor("Wk", (DH, DK), F32, kind="ExternalInput").ap()
    wv = nc.dram_tensor("Wv", (DH, DK), F32, kind="ExternalInput").ap()
    out = nc.dram_tensor("out", (S, BPC, DK), F32, kind="ExternalOutput").ap()
    with tile.TileContext(nc) as tc:
        _attention_kernel(tc, out, x1, x2, wq, wk, wv)
    nc.compile()
    _NC_CACHE = nc
    return nc


def _in_maps(x_1, x_2, Wq, Wk, Wv):
    maps = []
    for c in range(NCORES):
        bsl = slice(c * BPC, (c + 1) * BPC)
        maps.append({
            "x_1": np.ascontiguousarray(x_1[:, bsl, :], dtype=np.float32),
            "x_2": np.ascontiguousarray(x_2[:, bsl, :], dtype=np.float32),
            "Wq": np.asarray(Wq, dtype=np.float32),
            "Wk": np.asarray(Wk, dtype=np.float32),
            "Wv": np.asarray(Wv, dtype=np.float32),
        })
    return maps


def run(x_1, x_2, Wq, Wk, Wv, **spmd_kwargs):
    nc = _build()
    res = run_bass_kernel_spmd(
        nc, _in_maps(x_1, x_2, Wq, Wk, Wv), core_ids=list(range(NCORES)), **spmd_kwargs
    )
    out = np.concatenate([res.results[c]["out"] for c in range(NCORES)], axis=1)
    return out, res


def kernel(x_1, x_2, Wq, Wk, Wv):
    out, _ = run(x_1, x_2, Wq, Wk, Wv)
    return out.astype(np.float32)


# revision 8
# speedup vs baseline: 2.5171x; 1.1372x over previous
"""Cross-attention Bass/Tile kernel for Trainium2, data-parallel over batch on 8 cores.

Problem (hardcoded): x_1 [2048,16,100], x_2 [2048,16,100], Wq/Wk/Wv [100,128], fp32.
  Q = x1 @ Wq; K = x2 @ Wk; V = x2 @ Wv  (per batch)
  out = softmax(Q K^T / sqrt(128)) @ V   -> [2048,16,128]

Sharding: batch dim split 8 ways (2 batches per core). Full inputs in, full output out.

Per-core dataflow (2 batches), bf16 matmul core:
  - x tiles loaded with gpsimd cast-DMA (fp32->bf16), PE-transposed (bf16) -> x^T
  - Q^T,K^T,V^T = W^T @ x^T bf16 matmuls (k/v on partitions); V tiles via PE transpose
  - per 512-chunk of s (software-pipelined: output tail of chunk i-1 is emitted
    during chunk i so PE never stalls on the DVE/ACT tail):
      S^T tiles [t=128,s=512] = K^T_tile.T @ Q^T_chunk  (bf16), pairs in [128,1024] psum
      E^T = exp(S^T/sqrt(dk)) on ACT -> bf16
      rowsum via DVE tree adds -> rowacc [128,512]
      O^T[v,s] += V_tile.T @ E^T_tile  (bf16, psum-accumulated)
      tail: O^T->SBUF (ACT), per 128-block: rowsum[s,1] ones-matmul, recip (DVE),
            PE-transpose O^T->O, normalize fused into ACT psum->SBUF copy, DMA out
"""

import sys

sys.path.insert(0, "/opt/trn_rl_repo")

import numpy as np

import concourse.bass as bass
import concourse.tile as tile
from concourse import bacc, mybir
from concourse.bass_utils import run_bass_kernel_spmd
from concourse.masks import make_identity

S = 2048          # sequence length (both sides)
B = 16            # total batch
DH = 100          # input feature dim
DK = 128          # head dim
NCORES = 8
BPC = B // NCORES  # batches per core
F32 = mybir.dt.float32
BF16 = mybir.dt.bfloat16
SCALE = 1.0 / float(np.sqrt(np.float32(DK)))

ST = S // 128     # 16 t-tiles of 128
NSC = S // 512    # 4 chunks of 512


def _attention_kernel(tc, out, x1, x2, wq, wk, wv):
    nc = tc.nc

    with (
        tc.tile_pool(name="const", bufs=1) as constp,
        tc.tile_pool(name="xn", bufs=18) as xnp,
        tc.tile_pool(name="xT", bufs=4) as xtp,
        tc.tile_pool(name="qk", bufs=4) as qkp,
        tc.tile_pool(name="vt", bufs=2) as vtp,
        tc.tile_pool(name="vp", bufs=2) as vp,
        tc.tile_pool(name="et", bufs=2) as etp,
        tc.tile_pool(name="row", bufs=2) as rowp,
        tc.tile_pool(name="rr", bufs=8) as rrp,
        tc.tile_pool(name="osb", bufs=3) as osbp,
        tc.tile_pool(name="ps_big", bufs=2, space="PSUM") as psb,
        tc.tile_pool(name="ps_ot", bufs=2, space="PSUM") as psot,
        tc.tile_pool(name="ps_sm", bufs=2, space="PSUM") as pssm,
    ):
        ident = constp.tile([128, 128], F32)
        make_identity(nc, ident)
        ident_bf = constp.tile([128, 128], BF16)
        nc.vector.tensor_copy(ident_bf, ident)
        ones_f32 = constp.tile([128, 1], F32)
        nc.vector.memset(ones_f32, 1.0)
        w_sbs = []
        for wap, wname in ((wq, "wq"), (wk, "wk"), (wv, "wv")):
            w_f32 = constp.tile([DH, DK], F32, name=f"{wname}_f32")
            nc.sync.dma_start(w_f32, wap)
            w_sb = constp.tile([DH, DK], BF16, name=f"{wname}_sb")
            nc.vector.tensor_copy(w_sb, w_f32)
            w_sbs.append(w_sb)
        wq_sb, wk_sb, wv_sb = w_sbs

        # ---- x^T [d, S] bf16 per (input, batch).
        # One [128, 2*DH] cast-DMA load covers both batches (contiguous rows);
        # the transpose input view for batch b is cols [b*DH, b*DH+128).
        # PE transposes (bf16) grouped 4-per-psum-tile -> one [128,512] copy.
        xTs = {}
        for src_i in (0, 1):
            for b in range(BPC):
                xTs[(src_i, b)] = xtp.tile(
                    [128, S], BF16, tag="xT", name=f"xT_{src_i}_{b}"
                )
        for src_i, src_ap in ((0, x1), (1, x2)):
            xns = []
            for st in range(ST):
                xn2 = xnp.tile([128, 256], BF16, tag="xn", name=f"xn_{src_i}_{st}")
                nc.gpsimd.dma_start(
                    xn2[:, : BPC * DH],
                    src_ap[st * 128:(st + 1) * 128, :, :],
                )
                xns.append(xn2)
            for b in range(BPC):
                for g in range(4):
                    psq = pssm.tile([128, 512], BF16, tag="sm",
                                    name=f"xq_{src_i}_{b}_{g}")
                    for j in range(4):
                        st = g * 4 + j
                        nc.tensor.transpose(
                            psq[:, j * 128:(j + 1) * 128],
                            xns[st][:, b * DH: b * DH + 128],
                            ident_bf,
                        )
                    nc.vector.tensor_copy(
                        xTs[(src_i, b)][:, g * 512:(g + 1) * 512], psq
                    )

        # ---- projections (bf16): Q^T, K^T, V^T [DK, S] per batch
        qTs, kTs, vas = {}, {}, {}
        for b in range(BPC):
            x1T = xTs[(0, b)]
            x2T = xTs[(1, b)]
            qT = qkp.tile([DK, S], BF16, tag="qk", name=f"qT_{b}")
            kT = qkp.tile([DK, S], BF16, tag="qk", name=f"kT_{b}")
            vTsb = vtp.tile([DK, S], BF16, tag="vt", name=f"vT_{b}")
            for dstT, w_sb, xT in (
                (qT, wq_sb, x1T), (kT, wk_sb, x2T), (vTsb, wv_sb, x2T)
            ):
                for c in range(NSC):
                    csl = slice(c * 512, (c + 1) * 512)
                    ps = psot.tile([DK, 512], F32, tag="ot", name=f"pj_{b}_{c}")
                    nc.tensor.matmul(ps, w_sb, xT[:DH, csl], start=True, stop=True)
                    nc.scalar.copy(dstT[:, csl], ps)
            # V tiles [t, v] via grouped bf16 PE transposes
            vall = vp.tile([128, S], BF16, tag="v", name=f"vall_{b}")
            for g in range(4):
                psq = pssm.tile([128, 512], BF16, tag="sm", name=f"vq_{b}_{g}")
                for j in range(4):
                    tt = g * 4 + j
                    nc.tensor.transpose(
                        psq[:, j * 128:(j + 1) * 128],
                        vTsb[:, tt * 128:(tt + 1) * 128],
                        ident_bf,
                    )
                nc.vector.tensor_copy(vall[:, g * 512:(g + 1) * 512], psq)
            qTs[b], kTs[b], vas[b] = qT, kT, vall

        # ---- attention: per (batch, 512-chunk); output tail pipelined by one
        def emit_tail(st_):
            b, sc, rowacc, otp = st_
            ot_sb = osbp.tile([128, 512], F32, tag="osb", name=f"otsb_{b}_{sc}")
            nc.scalar.copy(ot_sb, otp)
            for si in range(4):
                scol = slice(si * 128, (si + 1) * 128)
                rs = pssm.tile([128, 1], F32, tag="sm", name=f"rs_{b}_{sc}_{si}")
                nc.tensor.matmul(rs, rowacc[:, scol], ones_f32,
                                 start=True, stop=True)
                rr = rrp.tile([128, 1], F32, tag="rr", name=f"rr_{b}_{sc}_{si}")
                nc.vector.reciprocal(rr, rs)
                otr = pssm.tile([128, 128], F32, tag="sm",
                                name=f"otr_{b}_{sc}_{si}")
                nc.tensor.transpose(otr, ot_sb[:, scol], ident)
                osc = osbp.tile([128, DK], F32, tag="osc", name=f"osc_{b}_{sc}_{si}")
                nc.scalar.mul(osc, otr, rr)
                s0 = sc * 512 + si * 128
                nc.sync.dma_start(out[s0:s0 + 128, b, :], osc)

        pending = None
        for b in range(BPC):
            qT, kT, vall = qTs[b], kTs[b], vas[b]
            for sc in range(NSC):
                ssl = slice(sc * 512, (sc + 1) * 512)
                # S^T tile pairs -> [128,1024] psum -> exp -> E^T bf16
                et_all = etp.tile([128, ST * 512], BF16, tag="et",
                                  name=f"et_{b}_{sc}")
                for tp2 in range(ST // 2):
                    ps = psb.tile([128, 1024], F32, tag="big",
                                  name=f"st_{b}_{sc}_{tp2}")
                    for h in range(2):
                        tt = tp2 * 2 + h
                        nc.tensor.matmul(
                            ps[:, h * 512:(h + 1) * 512],
                            kT[:, tt * 128:(tt + 1) * 128],
                            qT[:, ssl],
                            start=True, stop=True,
                        )
                    nc.scalar.activation(
                        et_all[:, tp2 * 1024:(tp2 + 1) * 1024], ps,
                        mybir.ActivationFunctionType.Exp, scale=SCALE,
                    )
                # rowsum: tree-structured DVE adds (wide views halve each level)
                acc = rowp.tile([128, 4096], F32, tag="acc", name=f"acc_{b}_{sc}")
                nc.vector.tensor_add(acc, et_all[:, :4096], et_all[:, 4096:])
                nc.vector.tensor_add(acc[:, :2048], acc[:, :2048], acc[:, 2048:])
                nc.vector.tensor_add(acc[:, :1024], acc[:, :1024], acc[:, 1024:2048])
                rowacc = acc[:, :512]
                nc.vector.tensor_add(rowacc, rowacc, acc[:, 512:1024])
                # O^T [v, s] accumulation over t tiles (bf16)
                otp = psot.tile([128, 512], F32, tag="ot", name=f"ot_{b}_{sc}")
                for tt in range(ST):
                    nc.tensor.matmul(
                        otp,
                        vall[:, tt * 128:(tt + 1) * 128],
                        et_all[:, tt * 512:(tt + 1) * 512],
                        start=(tt == 0),
                        stop=(tt == ST - 1),
                    )
                if pending is not None:
                    emit_tail(pending)
                pending = (b, sc, rowacc, otp)
        emit_tail(pending)


_NC_CACHE = None


def _build():
    global _NC_CACHE
    if _NC_CACHE is not None:
        return _NC_CACHE
    nc = bacc.Bacc("TRN2", target_bir_lowering=False, debug=False, num_devices=NCORES)
    x1 = nc.dram_tensor("x_1", (S, BPC, DH), F32, kind="ExternalInput").ap()
    x2 = nc.dram_tensor("x_2", (S, BPC, DH), F32, kind="ExternalInput").ap()
    wq = nc.dram_tensor("Wq", (DH, DK), F32, kind="ExternalInput").ap()
    wk = nc.dram_tensor("Wk", (DH, DK), F32, kind="ExternalInput").ap()
    wv = nc.dram_tensor("Wv", (DH, DK), F32, kind="ExternalInput").ap()
    out = nc.dram_tensor("out", (S, BPC, DK), F32, kind="ExternalOutput").ap()
    with tile.TileContext(nc) as tc:
        _attention_kernel(tc, out, x1, x2, wq, wk, wv)
    nc.compile()
    _NC_CACHE = nc
    return nc


def _in_maps(x_1, x_2, Wq, Wk, Wv):
    maps = []
    for c in range(NCORES):
        bsl = slice(c * BPC, (c + 1) * BPC)
        maps.append({
            "x_1": np.ascontiguousarray(x_1[:, bsl, :], dtype=np.float32),
            "x_2": np.ascontiguousarray(x_2[:, bsl, :], dtype=np.float32),
            "Wq": np.asarray(Wq, dtype=np.float32),
            "Wk": np.asarray(Wk, dtype=np.float32),
            "Wv": np.asarray(Wv, dtype=np.float32),
        })
    return maps


def run(x_1, x_2, Wq, Wk, Wv, **spmd_kwargs):
    nc = _build()
    res = run_bass_kernel_spmd(
        nc, _in_maps(x_1, x_2, Wq, Wk, Wv), core_ids=list(range(NCORES)), **spmd_kwargs
    )
    out = np.concatenate([res.results[c]["out"] for c in range(NCORES)], axis=1)
    return out, res


def kernel(x_1, x_2, Wq, Wk, Wv):
    out, _ = run(x_1, x_2, Wq, Wk, Wv)
    return out.astype(np.float32)


# revision 9
# speedup vs baseline: 2.6288x; 1.0444x over previous
"""Cross-attention Bass/Tile kernel for Trainium2, data-parallel over batch on 8 cores.

Problem (hardcoded): x_1 [2048,16,100], x_2 [2048,16,100], Wq/Wk/Wv [100,128], fp32.
  Q = x1 @ Wq; K = x2 @ Wk; V = x2 @ Wv  (per batch)
  out = softmax(Q K^T / sqrt(128)) @ V   -> [2048,16,128]

Sharding: batch dim split 8 ways (2 batches per core). Full inputs in, full output out.

Per-core dataflow (2 batches), bf16 matmul core:
  - x tiles loaded with gpsimd cast-DMA (fp32->bf16), PE-transposed (bf16) -> x^T
  - Q^T,K^T,V^T = W^T @ x^T bf16 matmuls (k/v on partitions); V tiles via PE transpose
  - per 512-chunk of s (software-pipelined: output tail of chunk i-1 is emitted
    during chunk i so PE never stalls on the DVE/ACT tail):
      S^T tiles [t=128,s=512] = K^T_tile.T @ Q^T_chunk  (bf16), pairs in [128,1024] psum
      E^T = exp(S^T/sqrt(dk)) on ACT -> bf16
      rowsum via DVE tree adds -> rowacc [128,512]
      O^T[v,s] += V_tile.T @ E^T_tile  (bf16, psum-accumulated)
      tail: O^T->SBUF (ACT), per 128-block: rowsum[s,1] ones-matmul, recip (DVE),
            PE-transpose O^T->O, normalize fused into ACT psum->SBUF copy, DMA out
"""

import sys

sys.path.insert(0, "/opt/trn_rl_repo")

import numpy as np

import concourse.bass as bass
import concourse.tile as tile
from concourse import bacc, mybir
from concourse.bass_utils import run_bass_kernel_spmd
from concourse.masks import make_identity

S = 2048          # sequence length (both sides)
B = 16            # total batch
DH = 100          # input feature dim
DK = 128          # head dim
NCORES = 8
BPC = B // NCORES  # batches per core
F32 = mybir.dt.float32
BF16 = mybir.dt.bfloat16
SCALE = 1.0 / float(np.sqrt(np.float32(DK)))

ST = S // 128     # 16 t-tiles of 128
NSC = S // 512    # 4 chunks of 512


def _attention_kernel(tc, out, x1, x2, wq, wk, wv):
    nc = tc.nc

    with (
        tc.tile_pool(name="const", bufs=1) as constp,
        tc.tile_pool(name="xn", bufs=18) as xnp,
        tc.tile_pool(name="xT", bufs=4) as xtp,
        tc.tile_pool(name="qk", bufs=4) as qkp,
        tc.tile_pool(name="vt", bufs=2) as vtp,
        tc.tile_pool(name="vp", bufs=2) as vp,
        tc.tile_pool(name="et", bufs=3) as etp,
        tc.tile_pool(name="row", bufs=2) as rowp,
        tc.tile_pool(name="rr", bufs=8) as rrp,
        tc.tile_pool(name="osb", bufs=3) as osbp,
        tc.tile_pool(name="ps_big", bufs=4, space="PSUM") as psb,
        tc.tile_pool(name="ps_ot", bufs=2, space="PSUM") as psot,
        tc.tile_pool(name="ps_sm", bufs=2, space="PSUM") as pssm,
    ):
        ident = constp.tile([128, 128], F32)
        make_identity(nc, ident)
        ident_bf = constp.tile([128, 128], BF16)
        nc.vector.tensor_copy(ident_bf, ident)
        ones_f32 = constp.tile([128, 1], F32)
        nc.vector.memset(ones_f32, 1.0)
        w_sbs = []
        for wap, wname in ((wq, "wq"), (wk, "wk"), (wv, "wv")):
            w_f32 = constp.tile([DH, DK], F32, name=f"{wname}_f32")
            nc.sync.dma_start(w_f32, wap)
            w_sb = constp.tile([DH, DK], BF16, name=f"{wname}_sb")
            nc.vector.tensor_copy(w_sb, w_f32)
            w_sbs.append(w_sb)
        wq_sb, wk_sb, wv_sb = w_sbs

        # ---- x^T [d, S] bf16 per (input, batch).
        # One [128, 2*DH] cast-DMA load covers both batches (contiguous rows);
        # the transpose input view for batch b is cols [b*DH, b*DH+128).
        # PE transposes (bf16) grouped 4-per-psum-tile -> one [128,512] copy.
        xTs = {}
        for src_i in (0, 1):
            for b in range(BPC):
                xTs[(src_i, b)] = xtp.tile(
                    [128, S], BF16, tag="xT", name=f"xT_{src_i}_{b}"
                )
        for src_i, src_ap in ((0, x1), (1, x2)):
            xns = []
            for st in range(ST):
                xn2 = xnp.tile([128, 256], BF16, tag="xn", name=f"xn_{src_i}_{st}")
                nc.gpsimd.dma_start(
                    xn2[:, : BPC * DH],
                    src_ap[st * 128:(st + 1) * 128, :, :],
                )
                xns.append(xn2)
            for b in range(BPC):
                for g in range(4):
                    psq = pssm.tile([128, 512], BF16, tag="sm",
                                    name=f"xq_{src_i}_{b}_{g}")
                    for j in range(4):
                        st = g * 4 + j
                        nc.tensor.transpose(
                            psq[:, j * 128:(j + 1) * 128],
                            xns[st][:, b * DH: b * DH + 128],
                            ident_bf,
                        )
                    nc.vector.tensor_copy(
                        xTs[(src_i, b)][:, g * 512:(g + 1) * 512], psq
                    )

        # ---- projections (bf16): Q^T, K^T, V^T [DK, S] per batch
        qTs, kTs, vas = {}, {}, {}
        for b in range(BPC):
            x1T = xTs[(0, b)]
            x2T = xTs[(1, b)]
            qT = qkp.tile([DK, S], BF16, tag="qk", name=f"qT_{b}")
            kT = qkp.tile([DK, S], BF16, tag="qk", name=f"kT_{b}")
            vTsb = vtp.tile([DK, S], BF16, tag="vt", name=f"vT_{b}")
            for dstT, w_sb, xT in (
                (qT, wq_sb, x1T), (kT, wk_sb, x2T), (vTsb, wv_sb, x2T)
            ):
                for c in range(NSC):
                    csl = slice(c * 512, (c + 1) * 512)
                    ps = psot.tile([DK, 512], F32, tag="ot", name=f"pj_{b}_{c}")
                    nc.tensor.matmul(ps, w_sb, xT[:DH, csl], start=True, stop=True)
                    nc.vector.tensor_copy(dstT[:, csl], ps)
            # V tiles [t, v] via grouped bf16 PE transposes
            vall = vp.tile([128, S], BF16, tag="v", name=f"vall_{b}")
            for g in range(4):
                psq = pssm.tile([128, 512], BF16, tag="sm", name=f"vq_{b}_{g}")
                for j in range(4):
                    tt = g * 4 + j
                    nc.tensor.transpose(
                        psq[:, j * 128:(j + 1) * 128],
                        vTsb[:, tt * 128:(tt + 1) * 128],
                        ident_bf,
                    )
                nc.vector.tensor_copy(vall[:, g * 512:(g + 1) * 512], psq)
            qTs[b], kTs[b], vas[b] = qT, kT, vall

        # ---- attention: per (batch, 512-chunk); output tail pipelined by one
        def emit_tail(st_):
            b, sc, rowacc, otp = st_
            ot_sb = osbp.tile([128, 512], F32, tag="osb", name=f"otsb_{b}_{sc}")
            nc.scalar.copy(ot_sb, otp)
            for si in range(4):
                scol = slice(si * 128, (si + 1) * 128)
                rs = pssm.tile([128, 1], F32, tag="sm", name=f"rs_{b}_{sc}_{si}")
                nc.tensor.matmul(rs, rowacc[:, scol], ones_f32,
                                 start=True, stop=True)
                rr = rrp.tile([128, 1], F32, tag="rr", name=f"rr_{b}_{sc}_{si}")
                nc.vector.reciprocal(rr, rs)
                otr = pssm.tile([128, 128], F32, tag="sm",
                                name=f"otr_{b}_{sc}_{si}")
                nc.tensor.transpose(otr, ot_sb[:, scol], ident)
                osc = osbp.tile([128, DK], F32, tag="osc", name=f"osc_{b}_{sc}_{si}")
                nc.vector.tensor_scalar_mul(osc, otr, rr)
                s0 = sc * 512 + si * 128
                nc.sync.dma_start(out[s0:s0 + 128, b, :], osc)

        pending = None
        for b in range(BPC):
            qT, kT, vall = qTs[b], kTs[b], vas[b]
            for sc in range(NSC):
                ssl = slice(sc * 512, (sc + 1) * 512)
                # S^T tile pairs -> [128,1024] psum -> exp -> E^T bf16
                et_all = etp.tile([128, ST * 512], BF16, tag="et",
                                  name=f"et_{b}_{sc}")
                for tt in range(ST):
                    ps = psb.tile([128, 512], F32, tag="big",
                                  name=f"st_{b}_{sc}_{tt}")
                    nc.tensor.matmul(
                        ps,
                        kT[:, tt * 128:(tt + 1) * 128],
                        qT[:, ssl],
                        start=True, stop=True,
                    )
                    nc.scalar.activation(
                        et_all[:, tt * 512:(tt + 1) * 512], ps,
                        mybir.ActivationFunctionType.Exp, scale=SCALE,
                    )
                # rowsum: tree-structured DVE adds (wide views halve each level)
                acch = rowp.tile([128, 4096], mybir.dt.float16, tag="acch",
                                 name=f"acch_{b}_{sc}")
                nc.vector.tensor_add(acch, et_all[:, :4096], et_all[:, 4096:])
                nc.vector.tensor_add(acch[:, :2048], acch[:, :2048], acch[:, 2048:])
                acc = rowp.tile([128, 1024], F32, tag="acc", name=f"acc_{b}_{sc}")
                nc.vector.tensor_add(acc, acch[:, :1024], acch[:, 1024:2048])
                rowacc = acc[:, :512]
                nc.vector.tensor_add(rowacc, rowacc, acc[:, 512:1024])
                # O^T [v, s] accumulation over t tiles (bf16)
                otp = psot.tile([128, 512], F32, tag="ot", name=f"ot_{b}_{sc}")
                for tt in range(ST):
                    nc.tensor.matmul(
                        otp,
                        vall[:, tt * 128:(tt + 1) * 128],
                        et_all[:, tt * 512:(tt + 1) * 512],
                        start=(tt == 0),
                        stop=(tt == ST - 1),
                    )
                if pending is not None:
                    emit_tail(pending)
                pending = (b, sc, rowacc, otp)
        emit_tail(pending)


_NC_CACHE = None


def _build():
    global _NC_CACHE
    if _NC_CACHE is not None:
        return _NC_CACHE
    nc = bacc.Bacc("TRN2", target_bir_lowering=False, debug=False, num_devices=NCORES)
    x1 = nc.dram_tensor("x_1", (S, BPC, DH), F32, kind="ExternalInput").ap()
    x2 = nc.dram_tensor("x_2", (S, BPC, DH), F32, kind="ExternalInput").ap()
    wq = nc.dram_tensor("Wq", (DH, DK), F32, kind="ExternalInput").ap()
    wk = nc.dram_tensor("Wk", (DH, DK), F32, kind="ExternalInput").ap()
    wv = nc.dram_tensor("Wv", (DH, DK), F32, kind="ExternalInput").ap()
    out = nc.dram_tensor("out", (S, BPC, DK), F32, kind="ExternalOutput").ap()
    with tile.TileContext(nc) as tc:
        _attention_kernel(tc, out, x1, x2, wq, wk, wv)
    nc.compile()
    _NC_CACHE = nc
    return nc


def _in_maps(x_1, x_2, Wq, Wk, Wv):
    maps = []
    for c in range(NCORES):
        bsl = slice(c * BPC, (c + 1) * BPC)
        maps.append({
            "x_1": np.ascontiguousarray(x_1[:, bsl, :], dtype=np.float32),
            "x_2": np.ascontiguousarray(x_2[:, bsl, :], dtype=np.float32),
            "Wq": np.asarray(Wq, dtype=np.float32),
            "Wk": np.asarray(Wk, dtype=np.float32),
            "Wv": np.asarray(Wv, dtype=np.float32),
        })
    return maps


def run(x_1, x_2, Wq, Wk, Wv, **spmd_kwargs):
    nc = _build()
    res = run_bass_kernel_spmd(
        nc, _in_maps(x_1, x_2, Wq, Wk, Wv), core_ids=list(range(NCORES)), **spmd_kwargs
    )
    out = np.concatenate([res.results[c]["out"] for c in range(NCORES)], axis=1)
    return out, res


def kernel(x_1, x_2, Wq, Wk, Wv):
    out, _ = run(x_1, x_2, Wq, Wk, Wv)
    return out.astype(np.float32)


# revision 10
# speedup vs baseline: 2.8008x; 1.0654x over previous
"""Cross-attention Bass/Tile kernel for Trainium2, data-parallel over batch on 8 cores.

Problem (hardcoded): x_1 [2048,16,100], x_2 [2048,16,100], Wq/Wk/Wv [100,128], fp32.
  Q = x1 @ Wq; K = x2 @ Wk; V = x2 @ Wv  (per batch)
  out = softmax(Q K^T / sqrt(128)) @ V   -> [2048,16,128]

Sharding: batch dim split 8 ways (2 batches per core). Full inputs in, full output out.

Per-core dataflow (2 batches), bf16 matmul core:
  - x tiles loaded with gpsimd cast-DMA (fp32->bf16), PE-transposed (bf16) -> x^T
  - Q^T,K^T,V^T = W^T @ x^T bf16 matmuls (k/v on partitions); V tiles via PE transpose
  - per 512-chunk of s (software-pipelined: output tail of chunk i-1 is emitted
    during chunk i so PE never stalls on the DVE/ACT tail):
      S^T tiles [t=128,s=512] = K^T_tile.T @ Q^T_chunk  (bf16), pairs in [128,1024] psum
      E^T = exp(S^T/sqrt(dk)) on ACT -> bf16
      rowsum via DVE tree adds -> rowacc [128,512]
      O^T[v,s] += V_tile.T @ E^T_tile  (bf16, psum-accumulated)
      tail: O^T->SBUF (ACT), per 128-block: rowsum[s,1] ones-matmul, recip (DVE),
            PE-transpose O^T->O, normalize fused into ACT psum->SBUF copy, DMA out
"""

import sys

sys.path.insert(0, "/opt/trn_rl_repo")

import numpy as np

import concourse.bass as bass
import concourse.tile as tile
from concourse import bacc, mybir
from concourse.bass_utils import run_bass_kernel_spmd
from concourse.masks import make_identity

S = 2048          # sequence length (both sides)
B = 16            # total batch
DH = 100          # input feature dim
DK = 128          # head dim
NCORES = 8
BPC = B // NCORES  # batches per core
F32 = mybir.dt.float32
BF16 = mybir.dt.bfloat16
SCALE = 1.0 / float(np.sqrt(np.float32(DK)))

ST = S // 128     # 16 t-tiles of 128
NSC = S // 512    # 4 chunks of 512


def _attention_kernel(tc, out, x1, x2, wq, wk, wv):
    nc = tc.nc

    with (
        tc.tile_pool(name="const", bufs=1) as constp,
        tc.tile_pool(name="xn", bufs=18) as xnp,
        tc.tile_pool(name="xT", bufs=4) as xtp,
        tc.tile_pool(name="qk", bufs=4) as qkp,
        tc.tile_pool(name="vp", bufs=2) as vp,
        tc.tile_pool(name="et", bufs=3) as etp,
        tc.tile_pool(name="row", bufs=2) as rowp,
        tc.tile_pool(name="rr", bufs=8) as rrp,
        tc.tile_pool(name="osb", bufs=3) as osbp,
        tc.tile_pool(name="ps_big", bufs=2, space="PSUM") as psb,
        tc.tile_pool(name="ps_ot", bufs=2, space="PSUM") as psot,
        tc.tile_pool(name="ps_sm", bufs=2, space="PSUM") as pssm,
    ):
        ident = constp.tile([128, 128], F32)
        make_identity(nc, ident)
        ident_bf = constp.tile([128, 128], BF16)
        nc.vector.tensor_copy(ident_bf, ident)
        ones_f32 = constp.tile([128, 1], F32)
        nc.vector.memset(ones_f32, 1.0)
        w_sbs = []
        for wap, wname in ((wq, "wq"), (wk, "wk"), (wv, "wv")):
            w_f32 = constp.tile([DH, DK], F32, name=f"{wname}_f32")
            nc.sync.dma_start(w_f32, wap)
            w_sb = constp.tile([DH, DK], BF16, name=f"{wname}_sb")
            nc.vector.tensor_copy(w_sb, w_f32)
            w_sbs.append(w_sb)
        wq_sb, wk_sb, wv_sb = w_sbs

        # ---- x^T [d, S] bf16 per (input, batch).
        # One [128, 2*DH] cast-DMA load covers both batches (contiguous rows);
        # the transpose input view for batch b is cols [b*DH, b*DH+128).
        # PE transposes (bf16) grouped 4-per-psum-tile -> one [128,512] copy.
        xTs = {}
        for src_i in (0, 1):
            for b in range(BPC):
                xTs[(src_i, b)] = xtp.tile(
                    [128, S], BF16, tag="xT", name=f"xT_{src_i}_{b}"
                )
        for src_i, src_ap in ((0, x1), (1, x2)):
            xns = []
            for st in range(ST):
                xn2 = xnp.tile([128, 256], BF16, tag="xn", name=f"xn_{src_i}_{st}")
                if src_i == 0:
                    nc.gpsimd.dma_start(
                        xn2[:, : BPC * DH],
                        src_ap[st * 128:(st + 1) * 128, :, :],
                    )
                else:
                    xf = xnp.tile([128, BPC * DH], F32, tag="xf",
                                  name=f"xf_{src_i}_{st}")
                    nc.sync.dma_start(xf, src_ap[st * 128:(st + 1) * 128, :, :])
                    nc.vector.tensor_copy(xn2[:, : BPC * DH], xf)
                xns.append(xn2)
            for b in range(BPC):
                for g in range(4):
                    psq = pssm.tile([128, 512], BF16, tag="sm",
                                    name=f"xq_{src_i}_{b}_{g}")
                    for j in range(4):
                        st = g * 4 + j
                        nc.tensor.transpose(
                            psq[:, j * 128:(j + 1) * 128],
                            xns[st][:, b * DH: b * DH + 128],
                            ident_bf,
                        )
                    nc.vector.tensor_copy(
                        xTs[(src_i, b)][:, g * 512:(g + 1) * 512], psq
                    )

        # ---- projections (bf16): Q^T, K^T, V^T [DK, S] per batch
        qTs, kTs, vas = {}, {}, {}
        for b in range(BPC):
            x1T = xTs[(0, b)]
            x2T = xTs[(1, b)]
            qT = qkp.tile([DK, S], BF16, tag="qk", name=f"qT_{b}")
            kT = qkp.tile([DK, S], BF16, tag="qk", name=f"kT_{b}")
            for dstT, w_sb, xT in ((qT, wq_sb, x1T), (kT, wk_sb, x2T)):
                for c in range(NSC):
                    csl = slice(c * 512, (c + 1) * 512)
                    ps = psot.tile([DK, 512], F32, tag="ot", name=f"pj_{b}_{c}")
                    nc.tensor.matmul(ps, w_sb, xT[:DH, csl], start=True, stop=True)
                    nc.vector.tensor_copy(dstT[:, csl], ps)
            # V tiles [t, v] directly: lhsT = x2^T slice, rhs = Wv; grouped
            # 4 matmuls into one [128,512] psum tile -> one copy
            vall = vp.tile([128, S], BF16, tag="v", name=f"vall_{b}")
            for g in range(4):
                psv = psot.tile([128, 512], F32, tag="ot", name=f"vg_{b}_{g}")
                for j in range(4):
                    tt = g * 4 + j
                    nc.tensor.matmul(
                        psv[:, j * 128:(j + 1) * 128],
                        x2T[:DH, tt * 128:(tt + 1) * 128],
                        wv_sb,
                        start=True, stop=True,
                    )
                nc.vector.tensor_copy(vall[:, g * 512:(g + 1) * 512], psv)
            qTs[b], kTs[b], vas[b] = qT, kT, vall

        # ---- attention: per (batch, 512-chunk); output tail pipelined by one
        def emit_tail(st_):
            b, sc, rowacc, otp = st_
            ot_sb = osbp.tile([128, 512], F32, tag="osb", name=f"otsb_{b}_{sc}")
            nc.scalar.copy(ot_sb, otp)
            for si in range(4):
                scol = slice(si * 128, (si + 1) * 128)
                rs = pssm.tile([128, 1], F32, tag="sm", name=f"rs_{b}_{sc}_{si}")
                nc.tensor.matmul(rs, rowacc[:, scol], ones_f32,
                                 start=True, stop=True)
                rr = rrp.tile([128, 1], F32, tag="rr", name=f"rr_{b}_{sc}_{si}")
                nc.vector.reciprocal(rr, rs)
                otr = pssm.tile([128, 128], F32, tag="sm",
                                name=f"otr_{b}_{sc}_{si}")
                nc.tensor.transpose(otr, ot_sb[:, scol], ident)
                osc = osbp.tile([128, DK], F32, tag="osc", name=f"osc_{b}_{sc}_{si}")
                nc.vector.tensor_scalar_mul(osc, otr, rr)
                s0 = sc * 512 + si * 128
                nc.sync.dma_start(out[s0:s0 + 128, b, :], osc)

        # Steady state: S^T matmuls of chunk i are ACT(exp)-gated via the two
        # [128,1024] psum slots; PV matmuls of chunk i-1 are interleaved into
        # the same PE stream to fill the gaps, and the chunk i-1 output tail
        # follows (its DVE/ACT deps are long since ready).
        items = [(b, sc) for b in range(BPC) for sc in range(NSC)]
        prev = None     # (b, sc, et_all, rowacc, otp_psum, vall)
        pending_tail = None
        for b, sc in items:
            qT, kT, vall = qTs[b], kTs[b], vas[b]
            ssl = slice(sc * 512, (sc + 1) * 512)
            et_all = etp.tile([128, ST * 512], BF16, tag="et", name=f"et_{b}_{sc}")
            if prev is not None:
                potp = psot.tile([128, 512], F32, tag="ot",
                                 name=f"ot_{prev[0]}_{prev[1]}")
            for tp2 in range(ST // 2):
                ps = psb.tile([128, 1024], F32, tag="big",
                              name=f"st_{b}_{sc}_{tp2}")
                for h in range(2):
                    tt = tp2 * 2 + h
                    nc.tensor.matmul(
                        ps[:, h * 512:(h + 1) * 512],
                        kT[:, tt * 128:(tt + 1) * 128],
                        qT[:, ssl],
                        start=True, stop=True,
                    )
                nc.scalar.activation(
                    et_all[:, tp2 * 1024:(tp2 + 1) * 1024], ps,
                    mybir.ActivationFunctionType.Exp, scale=SCALE,
                )
                if prev is not None:
                    pb, psc, pet, prow, pvall = prev
                    for h in range(2):
                        ptt = tp2 * 2 + h
                        nc.tensor.matmul(
                            potp,
                            pvall[:, ptt * 128:(ptt + 1) * 128],
                            pet[:, ptt * 512:(ptt + 1) * 512],
                            start=(ptt == 0),
                            stop=(ptt == ST - 1),
                        )
            # rowsum: tree adds (fp16 intermediate levels for 2x DVE mode)
            acch = rowp.tile([128, 4096], mybir.dt.float16, tag="acch",
                             name=f"acch_{b}_{sc}")
            nc.vector.tensor_add(acch, et_all[:, :4096], et_all[:, 4096:])
            nc.vector.tensor_add(acch[:, :2048], acch[:, :2048], acch[:, 2048:])
            acc = rowp.tile([128, 1024], F32, tag="acc", name=f"acc_{b}_{sc}")
            nc.vector.tensor_add(acc, acch[:, :1024], acch[:, 1024:2048])
            rowacc = acc[:, :512]
            nc.vector.tensor_add(rowacc, rowacc, acc[:, 512:1024])
            if pending_tail is not None:
                emit_tail(pending_tail)
            if prev is not None:
                pending_tail = (prev[0], prev[1], prev[3], potp)
            prev = (b, sc, et_all, rowacc, vall)
        # drain: PV + tail of the last chunk
        pb, psc, pet, prow, pvall = prev
        potp = psot.tile([128, 512], F32, tag="ot", name=f"ot_{pb}_{psc}")
        for ptt in range(ST):
            nc.tensor.matmul(
                potp,
                pvall[:, ptt * 128:(ptt + 1) * 128],
                pet[:, ptt * 512:(ptt + 1) * 512],
                start=(ptt == 0),
                stop=(ptt == ST - 1),
            )
        if pending_tail is not None:
            emit_tail(pending_tail)
        emit_tail((pb, psc, prow, potp))


_NC_CACHE = None


def _build():
    global _NC_CACHE
    if _NC_CACHE is not None:
        return _NC_CACHE
    nc = bacc.Bacc("TRN2", target_bir_lowering=False, debug=False, num_devices=NCORES)
    x1 = nc.dram_tensor("x_1", (S, BPC, DH), F32, kind="ExternalInput").ap()
    x2 = nc.dram_tensor("x_2", (S, BPC, DH), F32, kind="ExternalInput").ap()
    wq = nc.dram_tensor("Wq", (DH, DK), F32, kind="ExternalInput").ap()
    wk = nc.dram_tensor("Wk", (DH, DK), F32, kind="ExternalInput").ap()
    wv = nc.dram_tensor("Wv", (DH, DK), F32, kind="ExternalInput").ap()
    out = nc.dram_tensor("out", (S, BPC, DK), F32, kind="ExternalOutput").ap()
    with tile.TileContext(nc) as tc:
        _attention_kernel(tc, out, x1, x2, wq, wk, wv)
    nc.compile()
    _NC_CACHE = nc
    return nc


def _in_maps(x_1, x_2, Wq, Wk, Wv):
    maps = []
    for c in range(NCORES):
        bsl = slice(c * BPC, (c + 1) * BPC)
        maps.append({
            "x_1": np.ascontiguousarray(x_1[:, bsl, :], dtype=np.float32),
            "x_2": np.ascontiguousarray(x_2[:, bsl, :], dtype=np.float32),
            "Wq": np.asarray(Wq, dtype=np.float32),
            "Wk": np.asarray(Wk, dtype=np.float32),
            "Wv": np.asarray(Wv, dtype=np.float32),
        })
    return maps


def run(x_1, x_2, Wq, Wk, Wv, **spmd_kwargs):
    nc = _build()
    res = run_bass_kernel_spmd(
        nc, _in_maps(x_1, x_2, Wq, Wk, Wv), core_ids=list(range(NCORES)), **spmd_kwargs
    )
    out = np.concatenate([res.results[c]["out"] for c in range(NCORES)], axis=1)
    return out, res


def kernel(x_1, x_2, Wq, Wk, Wv):
    out, _ = run(x_1, x_2, Wq, Wk, Wv)
    return out.astype(np.float32)
